# revision 1
# baseline (speedup 1.0000x reference)
"""Trainium2 kernel for nn_AttentionBlock_37761352466542.

Contract: kernel(**inputs) takes FULL unsharded numpy inputs
(input_x: (4,3,256,256) f32, params: dict of weights) and returns the
FULL (4,2048,8,8) f32 output, computing on the 8 NeuronCores.

Distribution strategy (per sharding_hint): data-parallel over batch.
B=4 images are sharded one-per-core over 4 cores; train-mode BatchNorm
batch statistics are exact via cross-core psum of (sum, sum_sq).
The local-window attention tail is batch-local, so it needs no
communication; the final channel softmax is also local.
"""

import os

os.environ.setdefault("NEURON_CC_FLAGS", "--auto-cast=none")

import numpy as np

K = 7        # attention window
NH = 4       # num heads
DK = 2048
EPS = 1e-5
NB = 4       # batch
NSHARD = 4   # batch shards (one image per core)

_compiled = None


def _build():
    import jax
    import jax.numpy as jnp
    from jax.sharding import Mesh, PartitionSpec as P
    try:
        from jax.experimental.shard_map import shard_map
    except Exception:
        from jax.shard_map import shard_map  # newer jax

    def _conv(x, w, pad):
        return jax.lax.conv_general_dilated(
            x, w, (1, 1), pad, dimension_numbers=("NCHW", "OIHW", "NCHW"))

    def _bn_relu(x, g, b):
        # train-mode BN over (B,H,W); cross-core exact via pmean over the
        # batch mesh axis (equal shard sizes -> pmean of moments is exact).
        m = jax.lax.pmean(jnp.mean(x, axis=(0, 2, 3)), "b")
        e2 = jax.lax.pmean(jnp.mean(x * x, axis=(0, 2, 3)), "b")
        v = e2 - m * m
        y = (x - m[None, :, None, None]) * jax.lax.rsqrt(v + EPS)[None, :, None, None]
        y = y * g[None, :, None, None] + b[None, :, None, None]
        return jax.nn.relu(y)

    def _double(x, p, n):
        x = _bn_relu(_conv(x, p[f"w{n}"], "SAME"), p[f"g{n}"], p[f"b{n}"])
        return _bn_relu(_conv(x, p[f"w{n+1}"], "SAME"), p[f"g{n+1}"], p[f"b{n+1}"])

    def _pool(x):
        return jax.lax.reduce_window(x, -jnp.inf, jax.lax.max,
                                     (1, 1, 2, 2), (1, 1, 2, 2), "VALID")

    def _fwd_local(input_x, params):
        x = _double(input_x, params, 0)
        for n in (2, 4, 6, 8, 10):
            x = _double(_pool(x), params, n)
        B, C, H, W = x.shape  # (1, 2048, 8, 8) per shard
        pl = (K - 1) // 2
        pr = K - 1 - pl
        px = jnp.pad(x, ((0, 0), (0, 0), (pl, pr), (pl, pr)))
        k = _conv(px, params["kw"], "VALID") + params["kb"][None, :, None, None]
        q = _conv(x, params["qw"], "VALID") + params["qb"][None, :, None, None]
        v = _conv(px, params["vw"], "VALID") + params["vb"][None, :, None, None]
        ih = jnp.arange(H)[:, None] + jnp.arange(K)[None, :]
        iw = jnp.arange(W)[:, None] + jnp.arange(K)[None, :]
        kp = k[:, :, ih[:, None, :, None], iw[None, :, None, :]]
        vp = v[:, :, ih[:, None, :, None], iw[None, :, None, :]]
        dkh = C // NH
        kp = kp.reshape(B, NH, H, W, dkh, K * K)
        vp = vp.reshape(B, NH, H, W, dkh, K * K)
        qh = q.reshape(B, NH, H, W, dkh, 1)
        qk = jnp.matmul(jnp.swapaxes(qh, 4, 5), kp).reshape(B, NH, H, W, K, K)
        qsum = jnp.sum(qh, axis=(4, 5))
        rel = jnp.sum(params["rel_h"], 0) + jnp.sum(params["rel_w"], 0)
        qk = qk + qsum[..., None, None] * rel
        w_ = jax.nn.softmax(qk.reshape(B, NH, H, W, 1, K * K), axis=-1)
        out = jnp.matmul(w_, jnp.swapaxes(vp, 4, 5))
        out = out.reshape(B, -1, H, W)
        return jax.nn.softmax(out, axis=1)

    devs = jax.devices()[:NSHARD]
    mesh = Mesh(np.asarray(devs), ("b",))
    fwd = shard_map(
        _fwd_local, mesh=mesh,
        in_specs=(P("b"), P()), out_specs=P("b"), check_rep=False)
    jit_fwd = jax.jit(fwd)
    return jit_fwd


def kernel(input_x, params):
    global _compiled
    import jax.numpy as jnp
    if _compiled is None:
        _compiled = _build()
    params_f = {k: jnp.asarray(np.asarray(v, np.float32)) for k, v in params.items()}
    out = _compiled(jnp.asarray(np.asarray(input_x, np.float32)), params_f)
    return np.asarray(out, np.float32)
